# revision 1
# baseline (speedup 1.0000x reference)
"""MultiHeadDiffAttention TRN2 kernel.

Sharding: 8 cores = 2 batches x 4 head-pairs. Core c handles batch c//4 and
heads {2g, 2g+1} where g = c%4. The 2 heads = 128 channels = exactly one
GroupNorm group, so GroupNorm is core-local. The final projection is computed
as a partial sum over the core's 128 channels; the host adds the 4 partials
per batch plus the output bias.

Layout: "channel-major" [channels(partitions), sequence(free)] everywhere.
  - host pre-transposes/packs x and weights into per-partition-contiguous
    layouts so each big input is a single DMA
  - q/k projections land as qT/kT [128(2 heads x 64hd), 2048]
  - scores are computed transposed: S_T[k, q] (keys on partitions) so the
    exp'd scores feed the attn@V matmul directly as the moving operand
  - v is produced token-major [s, hd] with a ones-column appended, so the
    attn@V matmul also yields the softmax denominator (row 64 of PSUM out)
  - softmax skips max-subtraction: scores are bounded (|s|<2 for this data
    distribution, exp is exact in fp32)
  - per-(head, attn) exp-weight sums U are staged to SBUF; the diff-attn
    combine runs once per head over the full [64, 2048] row

The execution environment charges a large fixed cost per instruction, so the
structure minimizes total instruction count: exp in [128, 2048] groups,
single packed DMAs, full-row combine, one packed output store.
"""

import os
import sys

sys.path.insert(0, "/opt/trn_rl_repo")

import numpy as np

import concourse.bacc as bacc
import concourse.bass as bass
import concourse.mybir as mybir
import concourse.tile as tile
from concourse.masks import make_identity
from concourse.bass_utils import run_bass_kernel_spmd

B, S, D = 2, 2048, 512
H = 8
G = 4
HD = D // H          # 64
CH = 2 * HD          # 128 channels per core (one GroupNorm group)
LAMBDA_INIT = 0.2
EPS = 1e-5
N_CORES = 8

QB = 512             # query block (matmul N)
NQB = S // QB        # 4
KB = 128             # key block (matmul M)
NKB = S // KB        # 16
KG = 4               # key blocks per exp group ([128, 2048] PSUM tile)
NKG = NKB // KG      # 4
SB = 128             # seq block for v / final matmul
NSB = S // SB        # 16

F32 = mybir.dt.float32
F32R = mybir.dt.float32r
BF16 = mybir.dt.bfloat16

USE_BF16 = os.environ.get("KERNEL_BF16", "0") == "1"
MMDT = BF16 if USE_BF16 else F32R
NWEIGHTS = 5

_CACHE = {}


def r(ap):
    """bitcast an fp32-typed AP to float32r (no-op for bf16 tiles)"""
    if USE_BF16:
        return ap
    return ap.bitcast(F32R)


def build_program(repeats=1):
    nc = bacc.Bacc("TRN2", target_bir_lowering=False, debug=False)

    mmdt_in = MMDT if USE_BF16 else F32

    # ---- external I/O (packed per-partition-contiguous host layouts) ----
    # xp[p, c, s] = x[b, s, 128c+p]
    d_xp = nc.declare_dram_parameter("xp", [128, 4 * S], mmdt_in, isOutput=False)
    # wp[p, w, c, m] = W_w[ch0+m, 128c+p]; w in (q1,k1,q2,k2,v); + owT at tail
    d_wp = nc.declare_dram_parameter("wp", [128, NWEIGHTS * 512 + D],
                                     mmdt_in, isOutput=False)
    # cp[p, :] = [k1b, k2b, gnw, gnb, neglam0, neglam1]
    d_cp = nc.declare_dram_parameter("cp", [CH, 6], F32, isOutput=False)
    # yp[p, sb, d] = y_part[128*sb+p, d]
    d_y = nc.declare_dram_parameter("y_part", [SB, NSB * D], F32, isOutput=True)

    with tile.TileContext(nc) as tc:
      for _rep in range(repeats):
        with (
            tc.tile_pool(name="consts", bufs=1) as consts,
            tc.tile_pool(name="qk", bufs=1) as qk_pool,
            tc.tile_pool(name="vaug", bufs=1) as vaug_pool,
        ):
            # ---- constants / packed inputs ----
            ones = consts.tile([128, 1], F32, tag="ones")
            nc.vector.memset(ones, 1.0)
            eps_t = consts.tile([1, 1], F32, tag="eps")
            nc.vector.memset(eps_t, EPS)
            cp = consts.tile([CH, 6], F32, tag="cp")
            nc.sync.dma_start(out=cp, in_=d_cp.ap())
            k1b, k2b = cp[:, 0:1], cp[:, 1:2]
            gnw, gnb = cp[:, 2:3], cp[:, 3:4]
            neglam = cp[:, 4:6]

            wt = consts.tile([128, NWEIGHTS, 4, CH], MMDT, tag="wt")
            owT = consts.tile([CH, D], MMDT, tag="owT")
            wp_ap = d_wp.ap() if USE_BF16 else d_wp.ap().bitcast(F32R)
            nc.sync.dma_start(
                out=wt,
                in_=wp_ap[:, 0:NWEIGHTS * 512].rearrange(
                    "p (w c m) -> p w c m", w=NWEIGHTS, c=4))
            nc.sync.dma_start(out=owT, in_=wp_ap[:, NWEIGHTS * 512:])
            WIDX = {"q1": 0, "k1": 1, "q2": 2, "k2": 3, "v": 4}

            # ---- projections: qT/kT [128, 2048] channel-major ----
            qk = {}
            with (
                tc.tile_pool(name="xtp", bufs=1) as xt_pool,
                tc.tile_pool(name="pj", bufs=1, space="PSUM") as pj_pool,
                tc.tile_pool(name="pv", bufs=4, space="PSUM") as pv_pool,
            ):
                xt = xt_pool.tile([128, 4, S], MMDT, tag="xt")
                nc.sync.dma_start(
                    out=xt, in_=d_xp.ap().rearrange("p (c s) -> p c s", c=4)
                    if USE_BF16 else
                    d_xp.ap().bitcast(F32R).rearrange("p (c s) -> p c s", c=4))

                for w, bias in (("k1", k1b), ("q1", None), ("k2", k2b),
                                ("q2", None)):
                    dst = qk_pool.tile([CH, S], MMDT, tag=w)
                    qk[w] = dst
                    ps = pj_pool.tile([CH, 4 * QB], F32, tag="pj", name="pj")
                    for qb in range(NQB):
                        for c in range(4):
                            nc.tensor.matmul(
                                ps[:, qb * QB:(qb + 1) * QB],
                                wt[:, WIDX[w], c, :],
                                xt[:, c, qb * QB:(qb + 1) * QB],
                                start=(c == 0),
                                stop=(c == 3),
                            )
                    if bias is not None:
                        nc.vector.tensor_scalar_add(dst, ps, bias)
                    else:
                        nc.vector.tensor_copy(dst, ps)

                # ---- v: projected channel-major like q/k (4 wide MMs),
                # then PE-transposed per 128-block into token-major va
                # [s, hd] with a ones column; slot 2*sb+h ----
                identf = consts.tile([SB, SB], F32, tag="identf")
                make_identity(nc, identf)
                ident = consts.tile([SB, SB], MMDT, tag="ident")
                nc.vector.tensor_copy(ident, identf)
                vT = qk_pool.tile([CH, S], MMDT, tag="vT")
                ps = pj_pool.tile([CH, 4 * QB], F32, tag="pj", name="pjv")
                for qb in range(NQB):
                    for c in range(4):
                        nc.tensor.matmul(
                            ps[:, qb * QB:(qb + 1) * QB],
                            wt[:, WIDX["v"], c, :],
                            xt[:, c, qb * QB:(qb + 1) * QB],
                            start=(c == 0),
                            stop=(c == 3),
                        )
                nc.vector.tensor_copy(vT, ps)
                va = vaug_pool.tile([SB, 2 * NSB, HD + 1], MMDT, tag="va")
                nc.vector.tensor_copy(
                    va[:, :, HD:HD + 1],
                    ones.to_broadcast((SB, 2 * NSB, 1)))
                for sb in range(NSB):
                    pst = pv_pool.tile([SB, SB], MMDT, tag="pv", name="pv")
                    nc.tensor.transpose(
                        pst, vT[:, sb * SB:(sb + 1) * SB], ident)
                    nc.vector.tensor_copy(
                        va[:, 2 * sb:2 * sb + 2, 0:HD],
                        pst.rearrange("p (h m) -> p h m", h=2))

            # ---- attention-phase pools (reuse the x-tile region) ----
            with (
                tc.tile_pool(name="upool", bufs=2) as u_pool,
                tc.tile_pool(name="ubig", bufs=1) as ubig_pool,
                tc.tile_pool(name="opool", bufs=1) as o_pool,
                tc.tile_pool(name="small", bufs=1) as small,
            ):
              # ---- attention ----
              oT = o_pool.tile([CH, S], F32, tag="oT")
              # U[h][attn]: [65, 2048] exp-weight sums staged from PSUM
              U = {(h, a): ubig_pool.tile([HD + 1, S], F32, tag=f"U{h}{a}",
                                          name=f"U{h}{a}")
                   for h in (0, 1) for a in (1, 2)}

              with (
                  tc.tile_pool(name="sc", bufs=1, space="PSUM") as sc_pool,
                  tc.tile_pool(name="av", bufs=1, space="PSUM") as av_pool,
              ):
                  for attn in (1, 2):
                      qT, kT = qk[f"q{attn}"], qk[f"k{attn}"]
                      for h in (0, 1):
                          hs = slice(h * HD, (h + 1) * HD)
                          av = av_pool.tile([HD + 1, NQB * QB], F32,
                                            tag="av", name="av")
                          for qb in range(NQB):
                              for kg in range(NKG):
                                  sct = sc_pool.tile([128, KG * QB], F32,
                                                     tag="sc", name="sc")
                                  for j in range(KG):
                                      kb = kg * KG + j
                                      nc.tensor.matmul(
                                          sct[:, j * QB:(j + 1) * QB],
                                          r(kT[hs, kb * KB:(kb + 1) * KB]),
                                          r(qT[hs, qb * QB:(qb + 1) * QB]),
                                          start=True, stop=True,
                                      )
                                  ut = u_pool.tile([128, KG * QB], MMDT, tag="u")
                                  nc.scalar.activation(
                                      out=ut, in_=sct,
                                      func=mybir.ActivationFunctionType.Exp,
                                      scale=1.0 / (HD ** 0.5),
                                  )
                                  for j in range(KG):
                                      kb = kg * KG + j
                                      nc.tensor.matmul(
                                          av[:, qb * QB:(qb + 1) * QB],
                                          r(va[:, 2 * kb + h, :]),
                                          r(ut[:, j * QB:(j + 1) * QB]),
                                          start=(kb == 0),
                                          stop=(kb == NKB - 1),
                                      )
                          # stage U (incl. denominator row) to SBUF
                          nc.vector.tensor_copy(U[(h, attn)], av)

              # ---- combine: o = U1/r1 - lam * U2/r2 (full rows) ----
              for h in (0, 1):
                  hs = slice(h * HD, (h + 1) * HD)
                  rr1 = small.tile([1, S], F32, tag="rr1", name="rr")
                  rr2 = small.tile([1, S], F32, tag="rr2", name="rr")
                  nc.vector.reciprocal(out=rr1, in_=U[(h, 1)][HD:HD + 1, :])
                  nc.vector.reciprocal(out=rr2, in_=U[(h, 2)][HD:HD + 1, :])
                  rb1 = small.tile([HD, S], F32, tag="rb1", name="rb")
                  rb2 = small.tile([HD, S], F32, tag="rb2", name="rb")
                  nc.gpsimd.partition_broadcast(rb1, rr1)
                  nc.gpsimd.partition_broadcast(rb2, rr2)
                  t1 = small.tile([HD, S], F32, tag="t1", name="t1")
                  nc.vector.tensor_mul(t1, U[(h, 1)][0:HD, :], rb1)
                  t2 = small.tile([HD, S], F32, tag="rb1", name="t2")
                  nc.vector.scalar_tensor_tensor(
                      out=t2, in0=U[(h, 2)][0:HD, :],
                      scalar=neglam[0:HD, h:h + 1], in1=rb2,
                      op0=mybir.AluOpType.mult,
                      op1=mybir.AluOpType.mult,
                  )
                  nc.vector.tensor_add(oT[hs, :], t1, t2)

              # ---- GroupNorm (whole [128, 2048] is one group) ----
              xn = o_pool.tile([CH, S], MMDT, tag="xn")
              with tc.tile_pool(name="stp", bufs=1, space="PSUM") as stp_pool:
                  nst = S // nc.vector.BN_STATS_FMAX
                  bstats = small.tile([CH, nst, nc.vector.BN_STATS_DIM], F32,
                                      tag="bstats")
                  for i in range(nst):
                      nc.vector.bn_stats(
                          out=bstats[:, i, :],
                          in_=oT[:, i * nc.vector.BN_STATS_FMAX:
                                 (i + 1) * nc.vector.BN_STATS_FMAX])
                  mv = small.tile([CH, nc.vector.BN_AGGR_DIM], F32, tag="mv")
                  nc.vector.bn_aggr(out=mv, in_=bstats)
                  # per-partition [mean, E[x^2]] -> partition-sum via matmul
                  s12 = small.tile([CH, 2], F32, tag="s12")
                  nc.vector.tensor_copy(s12[:, 0:1], mv[:, 0:1])
                  nc.vector.scalar_tensor_tensor(
                      out=s12[:, 1:2], in0=mv[:, 0:1], scalar=0.0,
                      in1=mv[:, 0:1], op0=mybir.AluOpType.add,
                      op1=mybir.AluOpType.mult)
                  nc.vector.tensor_add(s12[:, 1:2], s12[:, 1:2], mv[:, 1:2])
                  st = stp_pool.tile([1, 2], F32, tag="st")
                  nc.tensor.matmul(st[0:1, 0:1], s12[:, 0:1], ones,
                                   start=True, stop=True)
                  nc.tensor.matmul(st[0:1, 1:2], s12[:, 1:2], ones,
                                   start=True, stop=True, skip_group_check=True)
                  mu_e2 = small.tile([1, 2], F32, tag="mu_e2")
                  nc.vector.tensor_scalar_mul(mu_e2, st[0:1, 0:2], 1.0 / CH)
                  sqm = small.tile([1, 1], F32, tag="sqm")
                  nc.vector.tensor_mul(sqm, mu_e2[:, 0:1], mu_e2[:, 0:1])
                  var = small.tile([1, 1], F32, tag="var")
                  nc.vector.tensor_sub(var, mu_e2[:, 1:2], sqm)
                  std = small.tile([1, 1], F32, tag="std")
                  nc.scalar.activation(out=std, in_=var,
                                       func=mybir.ActivationFunctionType.Sqrt,
                                       bias=eps_t, scale=1.0)
                  rstd = small.tile([1, 1], F32, tag="rstd")
                  nc.vector.reciprocal(out=rstd, in_=std)
                  murstd = small.tile([1, 2], F32, tag="murstd")
                  nc.vector.tensor_copy(murstd[:, 0:1], mu_e2[:, 0:1])
                  nc.vector.tensor_copy(murstd[:, 1:2], rstd)
                  br = small.tile([CH, 2], F32, tag="br")
                  nc.gpsimd.partition_broadcast(br, murstd)
                  a_t = small.tile([CH, 1], F32, tag="a_t")
                  nc.vector.tensor_mul(a_t, br[:, 1:2], gnw)
                  amu = small.tile([CH, 1], F32, tag="amu")
                  nc.vector.tensor_mul(amu, a_t, br[:, 0:1])
                  b_t = small.tile([CH, 1], F32, tag="b_t")
                  nc.vector.tensor_sub(b_t, gnb, amu)
                  nc.vector.tensor_scalar(out=xn, in0=oT, scalar1=a_t,
                                          scalar2=b_t,
                                          op0=mybir.AluOpType.mult,
                                          op1=mybir.AluOpType.add)

              # ---- final projection partial: y = xn.T @ owT ----
              # 4 MMs fill a [128, 2048] (4-bank) PSUM tile -> 1 copy each
              with tc.tile_pool(name="fin", bufs=2, space="PSUM") as fin_pool:
                  half = NSB // 4
                  for hf in range(4):
                      ps = fin_pool.tile([SB, half * D], F32, tag="fin",
                                         name="fin")
                      yt = o_pool.tile([SB, half, D], F32, tag="yt", name="yt")
                      for i in range(half):
                          sb = hf * half + i
                          nc.tensor.matmul(
                              ps[:, i * D:(i + 1) * D],
                              r(xn[:, sb * SB:(sb + 1) * SB]),
                              r(owT),
                              start=True, stop=True,
                          )
                      nc.vector.tensor_copy(yt, ps.rearrange(
                          "p (i d) -> p i d", i=half))
                      nc.sync.dma_start(
                          out=d_y.ap().rearrange(
                              "p (hf sb d) -> p hf sb d", hf=4, sb=half)[:, hf],
                          in_=yt)

    nc.compile()
    return nc


def _shard_inputs(inputs):
    import ml_dtypes
    mmnp = ml_dtypes.bfloat16 if USE_BF16 else np.float32
    x = np.ascontiguousarray(inputs["x"], np.float32)
    lam = (np.exp(inputs["lambda_q1"] * inputs["lambda_k1"])
           - np.exp(inputs["lambda_q2"] * inputs["lambda_k2"])
           + LAMBDA_INIT).astype(np.float32).reshape(H)
    in_maps = []
    for c in range(N_CORES):
        b, g = divmod(c, 4)
        ch = slice(CH * g, CH * (g + 1))
        # xp[p, c, s] = x[b, s, 128c+p]
        xp = np.ascontiguousarray(
            x[b].T.reshape(4, 128, S).transpose(1, 0, 2).reshape(128, 4 * S)
        ).astype(mmnp)
        wlist = []
        for W in (inputs["Q1_w"], inputs["K1_w"], inputs["Q2_w"],
                  inputs["K2_w"], inputs["V_w"]):
            wT = np.asarray(W)[ch].T  # [512, 128]
            wlist.append(np.ascontiguousarray(
                wT.reshape(4, 128, CH).transpose(1, 0, 2).reshape(128, 512)))
        owT = np.ascontiguousarray(np.asarray(inputs["out_w"])[:, ch].T)
        wp = np.concatenate(wlist + [owT], axis=1).astype(mmnp)
        cp = np.stack([
            np.asarray(inputs["K1_b"])[ch],
            np.asarray(inputs["K2_b"])[ch],
            np.asarray(inputs["gn_w"])[ch],
            np.asarray(inputs["gn_b"])[ch],
            np.full(CH, -lam[2 * g], np.float32),
            np.full(CH, -lam[2 * g + 1], np.float32),
        ], axis=1).astype(np.float32)
        in_maps.append({"xp": xp, "wp": wp, "cp": np.ascontiguousarray(cp)})
    return in_maps


def kernel(**inputs):
    inputs = {k: np.asarray(v) for k, v in inputs.items()}
    if "nc" not in _CACHE:
        _CACHE["nc"] = build_program()
    nc = _CACHE["nc"]
    in_maps = _shard_inputs(inputs)
    res = run_bass_kernel_spmd(nc, in_maps, list(range(N_CORES)))
    out_b = np.asarray(inputs["out_b"], np.float32)
    y = np.zeros((B, S, D), np.float32)
    for c in range(N_CORES):
        b = c // 4
        yp = res.results[c]["y_part"].astype(np.float32)
        y[b] += yp.reshape(SB, NSB, D).transpose(1, 0, 2).reshape(S, D)
    y += out_b[None, None, :]
    return y



# revision 4
# speedup vs baseline: 210.9843x; 210.9843x over previous
"""MultiHeadDiffAttention TRN2 kernel, v2 (pipelined).

Sharding: 8 cores = 2 batches x 4 head-pairs (same as v1). Core c handles
batch c//4 and heads {2g, 2g+1}, g = c%4; its 128 channels form one
GroupNorm group. Final projection is a partial sum over the core's
channels; the host adds 4 partials per batch plus the bias.

v2 structure (vs v1):
  - inputs/weights/outputs in bf16 (halves DMA bytes); attention math in
    fp32r, projections/final matmul in bf16
  - x DMA split into 4 chunks so projections start ~4x earlier
  - ALL psum use during proj+attention goes through two pools that coexist
    within the 8 banks: sc (2 tiles x 2 banks, double-buffered) and av
    (1 x 4 banks)
  - attention pipelined: per key-block, score MMs -> exp -> (deferred one
    group) attn@V MMs, so PE never waits on the activation engine
  - loops ordered h-outer/attn-inner; the diff-attn combine and GroupNorm
    stats for head-pair h run on DVE/Pool while h+1's attention occupies
    PE/ACT
  - exp groups are [128, 1024] (2 key-blocks x 512 queries... actually
    1 key-block x 2 query-chunks, keeping the stationary operand resident
    across consecutive matmuls)
"""

import sys

sys.path.insert(0, "/opt/trn_rl_repo")

import numpy as np

import concourse.bacc as bacc
import concourse.mybir as mybir
import concourse.tile as tile
from concourse.masks import make_identity
from concourse.bass_utils import run_bass_kernel_spmd

B, S, D = 2, 2048, 512
H = 8
HD = D // H          # 64
CH = 2 * HD          # 128 channels per core (one GroupNorm group)
LAMBDA_INIT = 0.2
EPS = 1e-5
N_CORES = 8

QB = 512             # query chunk (psum bank)
NQB = S // QB        # 4
KB = 128             # key block
NKB = S // KB        # 16
SB = 128             # seq block for final matmul
NSB = S // SB        # 16

F32 = mybir.dt.float32
F32R = mybir.dt.float32r
BF16 = mybir.dt.bfloat16
NWEIGHTS = 5
WIDX = {"q1": 0, "k1": 1, "q2": 2, "k2": 3, "v": 4}

_CACHE = {}

from contextlib import nullcontext


def build_program(repeats=1, hw_loop=False):
    nc = bacc.Bacc("TRN2", target_bir_lowering=False, debug=False)

    # ---- external I/O (packed per-partition-contiguous host layouts) ----
    # xp[p, c*S + s] = x[b, s, 128c+p]              (bf16)
    d_xp = nc.declare_dram_parameter("xp", [128, 4 * S], BF16, isOutput=False)
    # wp[p, w*512 + c*128 + m] = W_w[ch0+m, 128c+p]; then owT[p, d] tail
    d_wp = nc.declare_dram_parameter("wp", [128, NWEIGHTS * 512 + D],
                                     BF16, isOutput=False)
    # cp[p, :] = [k1b, k2b, gnw, gnb, neglam0, neglam1]
    d_cp = nc.declare_dram_parameter("cp", [CH, 6], F32, isOutput=False)
    # yp[p, sb*D + d] = y_part[128*sb+p, d]         (bf16)
    d_y = nc.declare_dram_parameter("y_part", [SB, NSB * D], BF16,
                                    isOutput=True)
    # yb[0, d] = (gn_b_eff . owT)[d] — constant row added host-side
    d_yb = nc.declare_dram_parameter("yb", [1, D], F32, isOutput=True)

    with tile.TileContext(nc) as tc:
     with (tc.For_i(0, repeats) if hw_loop else nullcontext()):
      for _rep in range(1 if hw_loop else repeats):
        with (
            tc.tile_pool(name="consts", bufs=1) as consts,
            tc.tile_pool(name="qk", bufs=1) as qk_pool,
            tc.tile_pool(name="vaug", bufs=1) as vaug_pool,
            tc.tile_pool(name="xtp", bufs=1) as xt_pool,
            tc.tile_pool(name="upool", bufs=3) as u_pool,
            tc.tile_pool(name="ubig", bufs=1) as ubig_pool,
            tc.tile_pool(name="opool", bufs=1) as o_pool,
            tc.tile_pool(name="small", bufs=1) as small,
        ):
            # ---- constants / packed inputs ----
            ones = consts.tile([128, 1], F32, tag="ones")
            nc.vector.memset(ones, 1.0)
            eps_t = consts.tile([1, 1], F32, tag="eps")
            nc.vector.memset(eps_t, EPS)
            cp = consts.tile([CH, 6], F32, tag="cp")
            nc.sync.dma_start(out=cp, in_=d_cp.ap())
            k1b, k2b = cp[:, 0:1], cp[:, 1:2]
            gnw, gnb = cp[:, 2:3], cp[:, 3:4]
            neglam = cp[:, 4:6]

            wt = consts.tile([128, NWEIGHTS, 4, CH], BF16, tag="wt")
            nc.sync.dma_start(
                out=wt,
                in_=d_wp.ap()[:, 0:NWEIGHTS * 512].rearrange(
                    "p (w c m) -> p w c m", w=NWEIGHTS, c=4))

            xt = xt_pool.tile([128, 4, S], BF16, tag="xt")
            for c in range(4):
                nc.sync.dma_start(out=xt[:, c, :],
                                  in_=d_xp.ap()[:, c * S:(c + 1) * S])

            owT = consts.tile([CH, D], BF16, tag="owT")
            nc.sync.dma_start(out=owT, in_=d_wp.ap()[:, NWEIGHTS * 512:])

            identf = consts.tile([SB, SB], F32, tag="identf")
            make_identity(nc, identf)
            ident = consts.tile([SB, SB], F32R, tag="ident")
            nc.vector.tensor_copy(ident, identf)

            # persistent SBUF tensors
            qk = {w: qk_pool.tile([CH, S], F32R, tag=w, name=w)
                  for w in ("q1", "k1", "q2", "k2")}
            vT = qk_pool.tile([CH, S], F32R, tag="vT")
            va = vaug_pool.tile([SB, 2 * NSB, HD + 1], F32R, tag="va")
            nc.vector.tensor_copy(va[:, :, HD:HD + 1],
                                  ones.to_broadcast((SB, 2 * NSB, 1)))
            oT = o_pool.tile([CH, S], F32, tag="oT")
            xnr = o_pool.tile([CH, S], BF16, tag="xnr")
            U = {(h, a): ubig_pool.tile([HD + 1, S], F32, tag=f"U{h}{a}",
                                         name=f"U{h}{a}")
                 for h in (0, 1) for a in (1, 2)}
            nst = 4
            BST_F = S // nst
            bstats = small.tile([CH, nst, 6], F32, tag="bstats")

            with (
                tc.tile_pool(name="sc", bufs=2, space="PSUM") as sc_pool,
                tc.tile_pool(name="avp", bufs=2, space="PSUM") as av_pool,
            ):
                # ---- projection of weight w into dst via the sc pool ----
                def project(w, dst, bias=None):
                    for half in range(2):        # 2 x [128, 1024] psum tiles
                        ps = sc_pool.tile([128, 2 * QB], F32, tag="sc",
                                          name=f"pj_{w}{half}")
                        for c in range(4):       # stationary wt[w,c] x2 MMs
                            for j in range(2):
                                qb = 2 * half + j
                                nc.tensor.matmul(
                                    ps[:, j * QB:(j + 1) * QB],
                                    wt[:, WIDX[w], c, :],
                                    xt[:, c, qb * QB:(qb + 1) * QB],
                                    start=(c == 0), stop=(c == 3),
                                )
                        sl = slice(2 * half * QB, 2 * (half + 1) * QB)
                        if bias is not None:
                            nc.vector.tensor_scalar_add(dst[:, sl], ps, bias)
                        else:
                            nc.vector.tensor_copy(dst[:, sl], ps)

                # v first: its transposes/copies overlap later projections
                project("v", vT)
                # transpose vT into token-major va, 4 key-blocks per psum tile
                for grp in range(4):
                    pst = sc_pool.tile([128, 4, SB], F32R, tag="sc",
                                       name=f"pv{grp}")
                    for i in range(4):
                        sb = 4 * grp + i
                        nc.tensor.transpose(
                            pst[:, i, :], vT[:, sb * SB:(sb + 1) * SB], ident)
                    nc.vector.tensor_copy(
                        va[:, 8 * grp:8 * grp + 8, 0:HD],
                        pst.rearrange("p i (h m) -> p (i h) m", h=2))

                project("q1", qk["q1"])
                project("k1", qk["k1"], k1b)

                # ---- attention for one (h, attn): pipelined sc->exp->av,
                # processed in two query-pair chunks so the combine for a
                # chunk overlaps the next chunk's attention ----
                def attention(h, attn):
                    hs = slice(h * HD, (h + 1) * HD)
                    qT, kT = qk[f"q{attn}"], qk[f"k{attn}"]
                    for pair in range(2):
                        pc = slice(2 * pair * QB, 2 * (pair + 1) * QB)
                        av = av_pool.tile([HD + 1, 2 * QB], F32, tag="av",
                                          name="av")
                        pending = None

                        def emit_av(p, av=av):
                            ut, kb = p
                            for j in range(2):
                                nc.tensor.matmul(
                                    av[:, j * QB:(j + 1) * QB],
                                    va[:, 2 * kb + h, :],
                                    ut[:, j * QB:(j + 1) * QB],
                                    start=(kb == 0), stop=(kb == NKB - 1),
                                )

                        for kb in range(NKB):
                            sct = sc_pool.tile([128, 2 * QB], F32, tag="sc",
                                               name="sc")
                            for j in range(2):
                                qb = 2 * pair + j
                                nc.tensor.matmul(
                                    sct[:, j * QB:(j + 1) * QB],
                                    kT[hs, kb * KB:(kb + 1) * KB],
                                    qT[hs, qb * QB:(qb + 1) * QB],
                                    start=True, stop=True,
                                )
                            ut = u_pool.tile([128, 2 * QB], F32R, tag="u")
                            nc.scalar.activation(
                                out=ut, in_=sct,
                                func=mybir.ActivationFunctionType.Exp,
                                scale=1.0 / (HD ** 0.5),
                            )
                            if pending is not None:
                                emit_av(pending)
                            pending = (ut, kb)
                        emit_av(pending)
                        nc.vector.tensor_copy(U[(h, attn)][:, pc], av)
                        if attn == 1:
                            combineA(h, pair)
                        else:
                            combineB(h, pair)

                # per-(h, query-pair) combine; runs on DVE/Pool under
                # the next chunk's attention. oT accumulates in place.
                t1s = {p: small.tile([HD, 2 * QB], F32, tag=f"t1_{p}",
                                     name=f"t1_{p}") for p in (0, 1)}

                def combineA(h, pair):
                    pc = slice(2 * pair * QB, 2 * (pair + 1) * QB)
                    rr = small.tile([1, 2 * QB], F32, tag="rr1", name="rr")
                    nc.vector.reciprocal(out=rr, in_=U[(h, 1)][HD:HD + 1, pc])
                    rb1 = small.tile([HD, 2 * QB], F32, tag="rb1", name="rb")
                    nc.gpsimd.partition_broadcast(rb1, rr)
                    nc.vector.tensor_mul(t1s[pair], U[(h, 1)][0:HD, pc], rb1)

                def combineB(h, pair):
                    hs = slice(h * HD, (h + 1) * HD)
                    pc = slice(2 * pair * QB, 2 * (pair + 1) * QB)
                    rr = small.tile([1, 2 * QB], F32, tag="rr2", name="rr")
                    nc.vector.reciprocal(out=rr, in_=U[(h, 2)][HD:HD + 1, pc])
                    rb2 = small.tile([HD, 2 * QB], F32, tag="rb2", name="rb")
                    nc.gpsimd.partition_broadcast(rb2, rr)
                    t2 = small.tile([HD, 2 * QB], F32, tag="t2", name="t2")
                    nc.vector.scalar_tensor_tensor(
                        out=t2, in0=U[(h, 2)][0:HD, pc],
                        scalar=neglam[0:HD, h:h + 1], in1=rb2,
                        op0=mybir.AluOpType.mult,
                        op1=mybir.AluOpType.mult,
                    )
                    nc.vector.tensor_add(oT[hs, pc], t1s[pair], t2)
                    for i in range(2):
                        ii = 2 * pair + i
                        nc.vector.bn_stats(
                            out=bstats[hs, ii, :],
                            in_=oT[hs, ii * BST_F:(ii + 1) * BST_F])
                    nc.vector.tensor_copy(xnr[hs, pc], oT[hs, pc])

                # ---- schedule: h-outer, attn-inner; q2/k2 proj overlaps ----
                attention(0, 1)
                project("q2", qk["q2"])
                project("k2", qk["k2"], k2b)
                attention(0, 2)
                attention(1, 1)
                attention(1, 2)

            # ---- GroupNorm global stats ----
            with tc.tile_pool(name="stp", bufs=1, space="PSUM") as stp_pool:
                mv = small.tile([CH, 2], F32, tag="mv")
                nc.vector.bn_aggr(out=mv, in_=bstats)
                s12 = small.tile([CH, 2], F32, tag="s12")
                nc.vector.tensor_copy(s12[:, 0:1], mv[:, 0:1])
                nc.vector.scalar_tensor_tensor(
                    out=s12[:, 1:2], in0=mv[:, 0:1], scalar=0.0,
                    in1=mv[:, 0:1], op0=mybir.AluOpType.add,
                    op1=mybir.AluOpType.mult)
                nc.vector.tensor_add(s12[:, 1:2], s12[:, 1:2], mv[:, 1:2])
                st = stp_pool.tile([1, 2], F32, tag="st")
                nc.tensor.matmul(st[0:1, 0:1], s12[:, 0:1], ones,
                                 start=True, stop=True)
                nc.tensor.matmul(st[0:1, 1:2], s12[:, 1:2], ones,
                                 start=True, stop=True, skip_group_check=True)
                mu_e2 = small.tile([1, 2], F32, tag="mu_e2")
                nc.vector.tensor_scalar_mul(mu_e2, st[0:1, 0:2], 1.0 / CH)
                sqm = small.tile([1, 1], F32, tag="sqm")
                nc.vector.tensor_mul(sqm, mu_e2[:, 0:1], mu_e2[:, 0:1])
                var = small.tile([1, 1], F32, tag="var")
                nc.vector.tensor_sub(var, mu_e2[:, 1:2], sqm)
                std = small.tile([1, 1], F32, tag="std")
                nc.scalar.activation(out=std, in_=var,
                                     func=mybir.ActivationFunctionType.Sqrt,
                                     bias=eps_t, scale=1.0)
                rstd = small.tile([1, 1], F32, tag="rstd")
                nc.vector.reciprocal(out=rstd, in_=std)
                murstd = small.tile([1, 2], F32, tag="murstd")
                nc.vector.tensor_copy(murstd[:, 0:1], mu_e2[:, 0:1])
                nc.vector.tensor_copy(murstd[:, 1:2], rstd)
                br = small.tile([CH, 2], F32, tag="br")
                nc.gpsimd.partition_broadcast(br, murstd)
                a_t = small.tile([CH, 1], F32, tag="a_t")
                nc.vector.tensor_mul(a_t, br[:, 1:2], gnw)
                amu = small.tile([CH, 1], F32, tag="amu")
                nc.vector.tensor_mul(amu, a_t, br[:, 0:1])
                b_t = small.tile([CH, 1], F32, tag="b_t")
                nc.vector.tensor_sub(b_t, gnb, amu)
                # fold GN affine into the output projection:
                #   y = xnr.T @ (a*owT) + (b.T @ owT)
                owTs = small.tile([CH, D], BF16, tag="owTs")
                nc.vector.tensor_scalar_mul(owTs, owT, a_t)
                b16 = small.tile([CH, 1], BF16, tag="b16")
                nc.vector.tensor_copy(b16, b_t)
                ybp = stp_pool.tile([1, D], F32, tag="ybp")
                nc.tensor.matmul(ybp, b16, owT, start=True, stop=True,
                                 skip_group_check=True)
                yb = small.tile([1, D], F32, tag="yb")
                nc.vector.tensor_copy(yb, ybp)
                nc.sync.dma_start(out=d_yb.ap(), in_=yb)

            # ---- final projection partial: y = xnr.T @ owTs ----
            with (
                tc.tile_pool(name="fin", bufs=2, space="PSUM") as fin_pool,
                tc.tile_pool(name="ytp", bufs=2) as yt_pool,
            ):
                half = NSB // 4
                for hf in range(4):
                    ps = fin_pool.tile([SB, half * D], F32, tag="fin",
                                       name="fin")
                    yt = yt_pool.tile([SB, half, D], BF16, tag="yt", name="yt")
                    for i in range(half):
                        sb = hf * half + i
                        nc.tensor.matmul(
                            ps[:, i * D:(i + 1) * D],
                            xnr[:, sb * SB:(sb + 1) * SB],
                            owTs,
                            start=True, stop=True,
                        )
                    nc.vector.tensor_copy(yt, ps.rearrange(
                        "p (i d) -> p i d", i=half))
                    nc.sync.dma_start(
                        out=d_y.ap().rearrange(
                            "p (hf sb d) -> p hf sb d", hf=4, sb=half)[:, hf],
                        in_=yt)

    nc.compile()
    return nc


def _shard_inputs(inputs):
    import ml_dtypes
    bf = ml_dtypes.bfloat16
    x = np.ascontiguousarray(inputs["x"], np.float32)
    lam = (np.exp(inputs["lambda_q1"] * inputs["lambda_k1"])
           - np.exp(inputs["lambda_q2"] * inputs["lambda_k2"])
           + LAMBDA_INIT).astype(np.float32).reshape(H)
    in_maps = []
    for c in range(N_CORES):
        b, g = divmod(c, 4)
        ch = slice(CH * g, CH * (g + 1))
        # xp[p, c*S+s] = x[b, s, 128c+p]
        xp = np.ascontiguousarray(
            x[b].T.reshape(4, 128, S).transpose(1, 0, 2).reshape(128, 4 * S)
        ).astype(bf)
        wlist = []
        for W in (inputs["Q1_w"], inputs["K1_w"], inputs["Q2_w"],
                  inputs["K2_w"], inputs["V_w"]):
            wT = np.asarray(W)[ch].T  # [512, 128]
            wlist.append(np.ascontiguousarray(
                wT.reshape(4, 128, CH).transpose(1, 0, 2).reshape(128, 512)))
        owT = np.ascontiguousarray(np.asarray(inputs["out_w"])[:, ch].T)
        wp = np.concatenate(wlist + [owT], axis=1).astype(bf)
        cp = np.stack([
            np.asarray(inputs["K1_b"])[ch],
            np.asarray(inputs["K2_b"])[ch],
            np.asarray(inputs["gn_w"])[ch],
            np.asarray(inputs["gn_b"])[ch],
            np.full(CH, -lam[2 * g], np.float32),
            np.full(CH, -lam[2 * g + 1], np.float32),
        ], axis=1).astype(np.float32)
        in_maps.append({"xp": xp, "wp": wp, "cp": np.ascontiguousarray(cp)})
    return in_maps


def kernel(**inputs):
    inputs = {k: np.asarray(v) for k, v in inputs.items()}
    if "nc" not in _CACHE:
        _CACHE["nc"] = build_program()
    nc = _CACHE["nc"]
    in_maps = _shard_inputs(inputs)
    res = run_bass_kernel_spmd(nc, in_maps, list(range(N_CORES)))
    out_b = np.asarray(inputs["out_b"], np.float32)
    y = np.zeros((B, S, D), np.float32)
    for c in range(N_CORES):
        b = c // 4
        yp = res.results[c]["y_part"].astype(np.float32)
        y[b] += yp.reshape(SB, NSB, D).transpose(1, 0, 2).reshape(S, D)
        y[b] += res.results[c]["yb"].astype(np.float32).reshape(1, D)
    y += out_b[None, None, :]
    return y


# revision 5
# speedup vs baseline: 214.9449x; 1.0188x over previous
"""MultiHeadDiffAttention TRN2 kernel, v2 (pipelined).

Sharding: 8 cores = 2 batches x 4 head-pairs (same as v1). Core c handles
batch c//4 and heads {2g, 2g+1}, g = c%4; its 128 channels form one
GroupNorm group. Final projection is a partial sum over the core's
channels; the host adds 4 partials per batch plus the bias.

v2 structure (vs v1):
  - inputs/weights/outputs in bf16 (halves DMA bytes); attention math in
    fp32r, projections/final matmul in bf16
  - x DMA split into 4 chunks so projections start ~4x earlier
  - ALL psum use during proj+attention goes through two pools that coexist
    within the 8 banks: sc (2 tiles x 2 banks, double-buffered) and av
    (1 x 4 banks)
  - attention pipelined: per key-block, score MMs -> exp -> (deferred one
    group) attn@V MMs, so PE never waits on the activation engine
  - loops ordered h-outer/attn-inner; the diff-attn combine and GroupNorm
    stats for head-pair h run on DVE/Pool while h+1's attention occupies
    PE/ACT
  - exp groups are [128, 1024] (2 key-blocks x 512 queries... actually
    1 key-block x 2 query-chunks, keeping the stationary operand resident
    across consecutive matmuls)
"""

import sys

sys.path.insert(0, "/opt/trn_rl_repo")

import numpy as np
from collections import deque

import concourse.bacc as bacc
import concourse.mybir as mybir
import concourse.tile as tile
from concourse.masks import make_identity
from concourse.bass_utils import run_bass_kernel_spmd

B, S, D = 2, 2048, 512
H = 8
HD = D // H          # 64
CH = 2 * HD          # 128 channels per core (one GroupNorm group)
LAMBDA_INIT = 0.2
EPS = 1e-5
N_CORES = 8

QB = 512             # query chunk (psum bank)
NQB = S // QB        # 4
KB = 128             # key block
NKB = S // KB        # 16
SB = 128             # seq block for final matmul
NSB = S // SB        # 16

F32 = mybir.dt.float32
F32R = mybir.dt.float32r
BF16 = mybir.dt.bfloat16
NWEIGHTS = 5
WIDX = {"q1": 0, "k1": 1, "q2": 2, "k2": 3, "v": 4}

_CACHE = {}

from contextlib import nullcontext


def build_program(repeats=1, hw_loop=False):
    nc = bacc.Bacc("TRN2", target_bir_lowering=False, debug=False)

    # ---- external I/O (packed per-partition-contiguous host layouts) ----
    # xp[p, c*S + s] = x[b, s, 128c+p]              (bf16)
    d_xp = nc.declare_dram_parameter("xp", [128, 4 * S], BF16, isOutput=False)
    # wp[p, w*512 + c*128 + m] = W_w[ch0+m, 128c+p]; then owT[p, d] tail
    d_wp = nc.declare_dram_parameter("wp", [128, NWEIGHTS * 512 + D],
                                     BF16, isOutput=False)
    # cp[p, :] = [k1b, k2b, gnw, gnb, neglam0, neglam1]
    d_cp = nc.declare_dram_parameter("cp", [CH, 6], F32, isOutput=False)
    # yp[p, sb*D + d] = y_part[128*sb+p, d]         (bf16)
    d_y = nc.declare_dram_parameter("y_part", [SB, NSB * D], BF16,
                                    isOutput=True)
    # yb[0, d] = (gn_b_eff . owT)[d] — constant row added host-side
    d_yb = nc.declare_dram_parameter("yb", [1, D], F32, isOutput=True)

    with tile.TileContext(nc) as tc:
     with (tc.For_i(0, repeats) if hw_loop else nullcontext()):
      for _rep in range(1 if hw_loop else repeats):
        with (
            tc.tile_pool(name="consts", bufs=1) as consts,
            tc.tile_pool(name="qk", bufs=1) as qk_pool,
            tc.tile_pool(name="vaug", bufs=1) as vaug_pool,
            tc.tile_pool(name="xtp", bufs=1) as xt_pool,
            tc.tile_pool(name="upool", bufs=9) as u_pool,
            tc.tile_pool(name="ubig", bufs=1) as ubig_pool,
            tc.tile_pool(name="opool", bufs=1) as o_pool,
            tc.tile_pool(name="small", bufs=1) as small,
        ):
            # ---- constants / packed inputs ----
            ones = consts.tile([128, 1], F32, tag="ones")
            nc.vector.memset(ones, 1.0)
            eps_t = consts.tile([1, 1], F32, tag="eps")
            nc.vector.memset(eps_t, EPS)
            cp = consts.tile([CH, 6], F32, tag="cp")
            nc.sync.dma_start(out=cp, in_=d_cp.ap())
            k1b, k2b = cp[:, 0:1], cp[:, 1:2]
            gnw, gnb = cp[:, 2:3], cp[:, 3:4]
            neglam = cp[:, 4:6]

            wt = consts.tile([128, NWEIGHTS, 4, CH], BF16, tag="wt")
            nc.sync.dma_start(
                out=wt,
                in_=d_wp.ap()[:, 0:NWEIGHTS * 512].rearrange(
                    "p (w c m) -> p w c m", w=NWEIGHTS, c=4))

            xt = xt_pool.tile([128, 4, S], BF16, tag="xt")
            for c in range(4):
                nc.sync.dma_start(out=xt[:, c, :],
                                  in_=d_xp.ap()[:, c * S:(c + 1) * S])

            owT = consts.tile([CH, D], BF16, tag="owT")
            nc.sync.dma_start(out=owT, in_=d_wp.ap()[:, NWEIGHTS * 512:])

            identf = consts.tile([SB, SB], F32, tag="identf")
            make_identity(nc, identf)
            ident = consts.tile([SB, SB], F32R, tag="ident")
            nc.vector.tensor_copy(ident, identf)

            # persistent SBUF tensors
            qk = {w: qk_pool.tile([CH, S], F32R, tag=w, name=w)
                  for w in ("q1", "k1", "q2", "k2")}
            vT = qk_pool.tile([CH, S], F32R, tag="vT")
            va = vaug_pool.tile([SB, 2 * NSB, HD + 1], F32R, tag="va")
            nc.vector.tensor_copy(va[:, :, HD:HD + 1],
                                  ones.to_broadcast((SB, 2 * NSB, 1)))
            oT = o_pool.tile([CH, S], F32, tag="oT")
            xnr = o_pool.tile([CH, S], BF16, tag="xnr")
            U = {(h, a): ubig_pool.tile([HD + 1, S], F32, tag=f"U{h}{a}",
                                         name=f"U{h}{a}")
                 for h in (0, 1) for a in (1, 2)}
            nst = 4
            BST_F = S // nst
            bstats = small.tile([CH, nst, 6], F32, tag="bstats")

            with (
                tc.tile_pool(name="sc", bufs=3, space="PSUM") as sc_pool,
                tc.tile_pool(name="avp", bufs=1, space="PSUM") as av_pool,
            ):
                # ---- projections & v-transpose as small chunks (PE
                # fillers interleaved into the first attention block) ----
                def proj_chunk(w, dst, qb, bias=None):
                    ps = sc_pool.tile([128, QB], F32, tag="sc",
                                      name=f"pj_{w}{qb}")
                    for c in range(4):
                        nc.tensor.matmul(
                            ps, wt[:, WIDX[w], c, :],
                            xt[:, c, qb * QB:(qb + 1) * QB],
                            start=(c == 0), stop=(c == 3))
                    sl = slice(qb * QB, (qb + 1) * QB)
                    if bias is not None:
                        nc.vector.tensor_scalar_add(dst[:, sl], ps, bias)
                    else:
                        nc.vector.tensor_copy(dst[:, sl], ps)

                def vtrans_chunk(grp):
                    pst = sc_pool.tile([128, 4, SB], F32R, tag="sc",
                                       name=f"pv{grp}")
                    for i in range(4):
                        sb = 4 * grp + i
                        nc.tensor.transpose(
                            pst[:, i, :], vT[:, sb * SB:(sb + 1) * SB], ident)
                    nc.vector.tensor_copy(
                        va[:, 8 * grp:8 * grp + 8, 0:HD],
                        pst.rearrange("p i (h m) -> p (i h) m", h=2))

                # ---- attention for one (h, attn): pipelined sc->exp->av,
                # processed in two query-pair chunks so the combine for a
                # chunk overlaps the next chunk's attention ----
                def attention(h, attn, fillers=None, fill_plan=None):
                    hs = slice(h * HD, (h + 1) * HD)
                    qT, kT = qk[f"q{attn}"], qk[f"k{attn}"]
                    for pair in range(2):
                        pc = slice(2 * pair * QB, 2 * (pair + 1) * QB)
                        av = av_pool.tile([HD + 1, 2 * QB], F32, tag="av",
                                          name="av")
                        pending = []

                        def emit_av(p, av=av):
                            ut, kb = p
                            for j in range(2):
                                nc.tensor.matmul(
                                    av[:, j * QB:(j + 1) * QB],
                                    va[:, 2 * kb + h, :],
                                    ut[:, j * QB:(j + 1) * QB],
                                    start=(kb == 0), stop=(kb == NKB - 1),
                                )

                        for kb in range(NKB):
                            sct = sc_pool.tile([128, 2 * QB], F32, tag="sc",
                                               name="sc")
                            for j in range(2):
                                qb = 2 * pair + j
                                nc.tensor.matmul(
                                    sct[:, j * QB:(j + 1) * QB],
                                    kT[hs, kb * KB:(kb + 1) * KB],
                                    qT[hs, qb * QB:(qb + 1) * QB],
                                    start=True, stop=True,
                                )
                            ut = u_pool.tile([128, 2 * QB], F32R, tag="u")
                            nc.scalar.activation(
                                out=ut, in_=sct,
                                func=mybir.ActivationFunctionType.Exp,
                                scale=1.0 / (HD ** 0.5),
                            )
                            pending.append((ut, kb))
                            if fill_plan is not None:
                                nfill, nflush = fill_plan(pair, kb)
                                for _ in range(nfill):
                                    if fillers:
                                        fillers.popleft()()
                                for _ in range(nflush):
                                    if len(pending) > 1:
                                        emit_av(pending.pop(0))
                            else:
                                if len(pending) > 1:
                                    emit_av(pending.pop(0))
                        for p in pending:
                            emit_av(p)
                        nc.vector.tensor_copy(U[(h, attn)][:, pc], av)
                        if attn == 1:
                            combineA(h, pair)
                        else:
                            combineB(h, pair)

                t1s = {p: small.tile([HD, 2 * QB], F32, tag=f"t1_{p}",
                                     name=f"t1_{p}") for p in (0, 1)}

                def combineA(h, pair):
                    pc = slice(2 * pair * QB, 2 * (pair + 1) * QB)
                    rr = small.tile([1, 2 * QB], F32, tag="rr1", name="rr")
                    nc.vector.reciprocal(out=rr, in_=U[(h, 1)][HD:HD + 1, pc])
                    rb1 = small.tile([HD, 2 * QB], F32, tag="rb1", name="rb")
                    nc.gpsimd.partition_broadcast(rb1, rr)
                    nc.vector.tensor_mul(t1s[pair], U[(h, 1)][0:HD, pc], rb1)

                def combineB(h, pair):
                    hs = slice(h * HD, (h + 1) * HD)
                    pc = slice(2 * pair * QB, 2 * (pair + 1) * QB)
                    rr = small.tile([1, 2 * QB], F32, tag="rr2", name="rr")
                    nc.vector.reciprocal(out=rr, in_=U[(h, 2)][HD:HD + 1, pc])
                    rb2 = small.tile([HD, 2 * QB], F32, tag="rb2", name="rb")
                    nc.gpsimd.partition_broadcast(rb2, rr)
                    t2 = small.tile([HD, 2 * QB], F32, tag="t2", name="t2")
                    nc.vector.scalar_tensor_tensor(
                        out=t2, in0=U[(h, 2)][0:HD, pc],
                        scalar=neglam[0:HD, h:h + 1], in1=rb2,
                        op0=mybir.AluOpType.mult,
                        op1=mybir.AluOpType.mult,
                    )
                    nc.vector.tensor_add(oT[hs, pc], t1s[pair], t2)
                    for i in range(2):
                        ii = 2 * pair + i
                        nc.vector.bn_stats(
                            out=bstats[hs, ii, :],
                            in_=oT[hs, ii * BST_F:(ii + 1) * BST_F])
                    nc.vector.tensor_copy(xnr[hs, pc], oT[hs, pc])

                # ---- schedule: a 3-chunk prologue starts the exp
                # stream ~3us in; every other projection chunk and the
                # v-transpose ride inside attention(0,1) as PE fillers;
                # attn@V for early key-blocks defers until transposed V
                # chunks exist. ----
                proj_chunk("k1", qk["k1"], 0, k1b)
                proj_chunk("q1", qk["q1"], 0)
                proj_chunk("q1", qk["q1"], 1)
                fillers = deque()
                for qb in (1, 2, 3):
                    fillers.append(lambda qb=qb: proj_chunk("k1", qk["k1"],
                                                            qb, k1b))
                for qb in range(4):
                    fillers.append(lambda qb=qb: proj_chunk("v", vT, qb))
                for g in range(4):
                    fillers.append(lambda g=g: vtrans_chunk(g))
                for qb in (2, 3):
                    fillers.append(lambda qb=qb: proj_chunk("q1", qk["q1"],
                                                            qb))
                for qb in range(4):
                    fillers.append(lambda qb=qb: proj_chunk("q2", qk["q2"],
                                                            qb))
                for qb in range(4):
                    fillers.append(lambda qb=qb: proj_chunk("k2", qk["k2"],
                                                            qb, k2b))

                def fill_plan(pair, kb):
                    it = pair * NKB + kb
                    if it <= 5:
                        return (2, 0)      # 11 head fillers by iter 5
                    if it <= 7:
                        return (1, 3)      # q1 tail + start AV flush
                    if it % 2 == 0:
                        return (1, 2)      # q2/k2 spread, keep flushing
                    return (0, 2)

                attention(0, 1, fillers, fill_plan)
                attention(0, 2)
                attention(1, 1)
                attention(1, 2)
                # preload the Sqrt activation table while DVE combines
                dummy = small.tile([1, 1], F32, tag="dummy")
                nc.scalar.activation(out=dummy, in_=eps_t,
                                     func=mybir.ActivationFunctionType.Sqrt,
                                     scale=1.0)

            # ---- GroupNorm global stats ----
            with tc.tile_pool(name="stp", bufs=1, space="PSUM") as stp_pool:
                mv = small.tile([CH, 2], F32, tag="mv")
                nc.vector.bn_aggr(out=mv, in_=bstats)
                s12 = small.tile([CH, 2], F32, tag="s12")
                nc.vector.tensor_copy(s12[:, 0:1], mv[:, 0:1])
                nc.vector.scalar_tensor_tensor(
                    out=s12[:, 1:2], in0=mv[:, 0:1], scalar=0.0,
                    in1=mv[:, 0:1], op0=mybir.AluOpType.add,
                    op1=mybir.AluOpType.mult)
                nc.vector.tensor_add(s12[:, 1:2], s12[:, 1:2], mv[:, 1:2])
                st = stp_pool.tile([1, 2], F32, tag="st")
                nc.tensor.matmul(st[0:1, 0:1], s12[:, 0:1], ones,
                                 start=True, stop=True)
                nc.tensor.matmul(st[0:1, 1:2], s12[:, 1:2], ones,
                                 start=True, stop=True, skip_group_check=True)
                mu_e2 = small.tile([1, 2], F32, tag="mu_e2")
                nc.vector.tensor_scalar_mul(mu_e2, st[0:1, 0:2], 1.0 / CH)
                sqm = small.tile([1, 1], F32, tag="sqm")
                nc.vector.tensor_mul(sqm, mu_e2[:, 0:1], mu_e2[:, 0:1])
                var = small.tile([1, 1], F32, tag="var")
                nc.vector.tensor_sub(var, mu_e2[:, 1:2], sqm)
                std = small.tile([1, 1], F32, tag="std")
                nc.scalar.activation(out=std, in_=var,
                                     func=mybir.ActivationFunctionType.Sqrt,
                                     bias=eps_t, scale=1.0)
                rstd = small.tile([1, 1], F32, tag="rstd")
                nc.vector.reciprocal(out=rstd, in_=std)
                murstd = small.tile([1, 2], F32, tag="murstd")
                nc.vector.tensor_copy(murstd[:, 0:1], mu_e2[:, 0:1])
                nc.vector.tensor_copy(murstd[:, 1:2], rstd)
                br = small.tile([CH, 2], F32, tag="br")
                nc.gpsimd.partition_broadcast(br, murstd)
                a_t = small.tile([CH, 1], F32, tag="a_t")
                nc.vector.tensor_mul(a_t, br[:, 1:2], gnw)
                amu = small.tile([CH, 1], F32, tag="amu")
                nc.vector.tensor_mul(amu, a_t, br[:, 0:1])
                b_t = small.tile([CH, 1], F32, tag="b_t")
                nc.vector.tensor_sub(b_t, gnb, amu)
                # fold GN affine into the output projection:
                #   y = xnr.T @ (a*owT) + (b.T @ owT)
                owTs = small.tile([CH, D], BF16, tag="owTs")
                nc.vector.tensor_scalar_mul(owTs, owT, a_t)
                b16 = small.tile([CH, 1], BF16, tag="b16")
                nc.vector.tensor_copy(b16, b_t)
                ybp = stp_pool.tile([1, D], F32, tag="ybp")
                nc.tensor.matmul(ybp, b16, owT, start=True, stop=True,
                                 skip_group_check=True)
                yb = small.tile([1, D], F32, tag="yb")
                nc.vector.tensor_copy(yb, ybp)
                nc.sync.dma_start(out=d_yb.ap(), in_=yb)

            # ---- final projection partial: y = xnr.T @ owTs ----
            with (
                tc.tile_pool(name="fin", bufs=2, space="PSUM") as fin_pool,
                tc.tile_pool(name="ytp", bufs=2) as yt_pool,
            ):
                half = NSB // 4
                for hf in range(4):
                    ps = fin_pool.tile([SB, half * D], F32, tag="fin",
                                       name="fin")
                    yt = yt_pool.tile([SB, half, D], BF16, tag="yt", name="yt")
                    for i in range(half):
                        sb = hf * half + i
                        nc.tensor.matmul(
                            ps[:, i * D:(i + 1) * D],
                            xnr[:, sb * SB:(sb + 1) * SB],
                            owTs,
                            start=True, stop=True,
                        )
                    nc.vector.tensor_copy(yt, ps.rearrange(
                        "p (i d) -> p i d", i=half))
                    nc.sync.dma_start(
                        out=d_y.ap().rearrange(
                            "p (hf sb d) -> p hf sb d", hf=4, sb=half)[:, hf],
                        in_=yt)

    nc.compile()
    return nc


def _shard_inputs(inputs):
    import ml_dtypes
    bf = ml_dtypes.bfloat16
    x = np.ascontiguousarray(inputs["x"], np.float32)
    lam = (np.exp(inputs["lambda_q1"] * inputs["lambda_k1"])
           - np.exp(inputs["lambda_q2"] * inputs["lambda_k2"])
           + LAMBDA_INIT).astype(np.float32).reshape(H)
    in_maps = []
    for c in range(N_CORES):
        b, g = divmod(c, 4)
        ch = slice(CH * g, CH * (g + 1))
        # xp[p, c*S+s] = x[b, s, 128c+p]
        xp = np.ascontiguousarray(
            x[b].T.reshape(4, 128, S).transpose(1, 0, 2).reshape(128, 4 * S)
        ).astype(bf)
        wlist = []
        for W in (inputs["Q1_w"], inputs["K1_w"], inputs["Q2_w"],
                  inputs["K2_w"], inputs["V_w"]):
            wT = np.asarray(W)[ch].T  # [512, 128]
            wlist.append(np.ascontiguousarray(
                wT.reshape(4, 128, CH).transpose(1, 0, 2).reshape(128, 512)))
        owT = np.ascontiguousarray(np.asarray(inputs["out_w"])[:, ch].T)
        wp = np.concatenate(wlist + [owT], axis=1).astype(bf)
        cp = np.stack([
            np.asarray(inputs["K1_b"])[ch],
            np.asarray(inputs["K2_b"])[ch],
            np.asarray(inputs["gn_w"])[ch],
            np.asarray(inputs["gn_b"])[ch],
            np.full(CH, -lam[2 * g], np.float32),
            np.full(CH, -lam[2 * g + 1], np.float32),
        ], axis=1).astype(np.float32)
        in_maps.append({"xp": xp, "wp": wp, "cp": np.ascontiguousarray(cp)})
    return in_maps


def kernel(**inputs):
    inputs = {k: np.asarray(v) for k, v in inputs.items()}
    if "nc" not in _CACHE:
        _CACHE["nc"] = build_program()
    nc = _CACHE["nc"]
    in_maps = _shard_inputs(inputs)
    res = run_bass_kernel_spmd(nc, in_maps, list(range(N_CORES)))
    out_b = np.asarray(inputs["out_b"], np.float32)
    y = np.zeros((B, S, D), np.float32)
    for c in range(N_CORES):
        b = c // 4
        yp = res.results[c]["y_part"].astype(np.float32)
        y[b] += yp.reshape(SB, NSB, D).transpose(1, 0, 2).reshape(S, D)
        y[b] += res.results[c]["yb"].astype(np.float32).reshape(1, D)
    y += out_b[None, None, :]
    return y
